# revision 1
# baseline (speedup 1.0000x reference)
"""Trainium2 Bass kernel for nn_HeatmapLayer: separable Gaussian heatmaps.

Reference math (per batch b, class c):
    mx = labels[b, 2c] * H ; my = labels[b, 2c+1] * W          (H = W = 384)
    sigma = H * exp(log_weight)
    dx2[h] = (h - mx)^2 / sigma        ; normalized by its min over h
    dy2[w] = (w - my)^2 / (20 * sigma) ; normalized by its min over w
    out[b,c,h,w] = exp(-0.5*(dx2[h] + dy2[w])) = ex[h] * ey[w]

Each (b,c) heatmap is a rank-1 outer product of two 384-length
profiles.  Per core (pure data parallel over batch: 2 batches = 12
(b,c) pairs per core).

Critical path to the first output DMA is kept short (few cross-engine
hops): the log-domain x-profile lxm (both min-normalization
corrections folded in) is computed with back-to-back ACT ops on a
[12, 2, 384] tile, PE-transposed (3 matmul-transposes), copied
PSUM->SBUF on ACT, and exponentiated once as a [128, 36] ACT op.

Two per-pair paths balance the Vector and Scalar engines (every
output element is written exactly once by one of them):

  * DVE path (9 pairs):  ey_p(w) = U(w) * exp(a_p*w + c_p) with
    U(w) = exp(sc_y*w^2) shared across pairs; per pair one ACT Exp,
    one DVE tensor_tensor (U*E_p), then 3 DVE tensor_scalar
    multiplies by EXT (the transposed x-profile).
  * ACT path (3 pairs):  one ACT Square -> sq_y, then per chunk one
    ACT Exp(sq_y*sc_y + LXT[:,c,p]) writes the final chunk directly.

Exp args stay within +-54, far from f32 limits, because
min (w-my)^2 <= 1 and sc_y*384^2 <= 54 for Xavier-bounded log_weight.

Output staged in SBUF, one ~576KB HWDGE DMA per pair (the ~16-20us
per-core DMA roofline).  x is only used for its shape; it is never
transferred to the device.
"""

import numpy as np
from contextlib import ExitStack

import concourse.bacc as bacc
import concourse.bass as bass
import concourse.tile as tile
from concourse import mybir
from concourse.bass_utils import run_bass_kernel_spmd
from concourse.masks import make_identity

B, CH, H, W = 16, 3, 384, 384
NCLS = 6
N_CORES = 8
BPC = B // N_CORES            # batches per core = 2
PAIRS = BPC * NCLS            # (b,c) pairs per core = 12
P = 128
CHUNKS = H // P               # 3
LN_H = float(np.log(H))
F32 = mybir.dt.float32
AF = mybir.ActivationFunctionType

ACT_PAIRS = set()             # all pairs on the DVE path (v4 balance)
# engine for the 36 output multiplies, by flat index (p*3+c)
MULT_ENGINE = ("vvs" * 8) + ("vsv" * 2) + ("vvv" * 2)


def build_bass() -> bass.Bass:
    nc = bacc.Bacc("TRN2", target_bir_lowering=False, debug=False,
                   num_devices=N_CORES)
    labels = nc.dram_tensor("labels", [BPC, 2 * NCLS], F32,
                            kind="ExternalInput")
    logw = nc.dram_tensor("log_weight", [1, 1], F32, kind="ExternalInput")
    out = nc.dram_tensor("out", [PAIRS * H, W], F32, kind="ExternalOutput")

    with ExitStack() as ctx:
        tc = ctx.enter_context(tile.TileContext(nc))
        singles = ctx.enter_context(tc.tile_pool(name="singles", bufs=1))
        psum = ctx.enter_context(tc.tile_pool(name="psum", bufs=3,
                                              space="PSUM"))
        ybuf = ctx.enter_context(tc.tile_pool(name="ybuf", bufs=4))
        stage = ctx.enter_context(tc.tile_pool(name="stage", bufs=6))

        # ---- shared grid: iota in f32 (0..383 exact) ---------------------
        iog = singles.tile([P, W], F32)
        nc.gpsimd.iota(iog, pattern=[[1, W]], base=0, channel_multiplier=0,
                       allow_small_or_imprecise_dtypes=True)

        # ---- x-profile chain (pairs on partitions 0..11) -----------------
        lab = singles.tile([PAIRS, 2], F32)
        nc.sync.dma_start(
            out=lab,
            in_=labels[:, :].rearrange("b (q two) -> (b q) two", two=2),
        )
        lwb = singles.tile([PAIRS, 1], F32)
        nc.gpsimd.dma_start(out=lwb, in_=logw[:, :].to_broadcast((PAIRS, 1)))

        # neg_m[:,0] = -mx, neg_m[:,1] = -my          (DVE, parallel)
        neg_m = singles.tile([PAIRS, 2], F32)
        nc.vector.tensor_scalar_mul(out=neg_m, in0=lab, scalar1=-float(H))

        # inv_s = 1/sigma = exp(-log_weight - ln(H))  (ACT, back-to-back)
        nlw = singles.tile([PAIRS, 1], F32)
        nc.vector.tensor_scalar(out=nlw, in0=lwb, scalar1=-1.0,
                                scalar2=-LN_H, op0=mybir.AluOpType.mult,
                                op1=mybir.AluOpType.add)
        inv_s = singles.tile([PAIRS, 1], F32)
        nc.scalar.activation(out=inv_s, in_=nlw, func=AF.Exp,
                             bias=0.0, scale=1.0)
        # sc columns: 0: -inv_s/2 (x exp scale), 1: +inv_s/2, 2: +inv_s/40
        sc = singles.tile([PAIRS, 3], F32)
        for i, m in enumerate((-0.5, 0.5, 0.025)):
            nc.vector.tensor_scalar_mul(out=sc[:, i:i + 1], in0=inv_s,
                                        scalar1=m)

        # both squared-distance profiles in one tile -> ONE min-reduce
        sqxy = singles.tile([PAIRS, 2, W], F32)
        nc.scalar.activation(out=sqxy[:, 0, :], in_=iog[:PAIRS, :],
                             func=AF.Square, bias=neg_m[:, 0:1], scale=1.0)
        nc.scalar.activation(out=sqxy[:, 1, :], in_=iog[:PAIRS, :],
                             func=AF.Square, bias=neg_m[:, 1:2], scale=1.0)
        mn2 = singles.tile([PAIRS, 2], F32)
        nc.vector.tensor_reduce(out=mn2, in_=sqxy, axis=mybir.AxisListType.X,
                                op=mybir.AluOpType.min)
        # b2 = inv_s/2 * min_x + inv_s/40 * min_y
        bb = singles.tile([PAIRS, 2], F32)
        nc.vector.tensor_mul(out=bb, in0=mn2, in1=sc[:, 1:3])
        b2 = singles.tile([PAIRS, 1], F32)
        nc.vector.tensor_reduce(out=b2, in_=bb, axis=mybir.AxisListType.X,
                                op=mybir.AluOpType.add)
        # log-domain x profile, on ACT (same engine as its consumer chain)
        lxm = singles.tile([PAIRS, W], F32)
        nc.scalar.activation(out=lxm, in_=sqxy[:, 0, :], func=AF.Identity,
                             bias=b2, scale=sc[:, 0:1])

        # ---- PE-transpose lxm; copies on ACT; exponentiate once ----------
        ident = singles.tile([PAIRS, PAIRS], F32)
        make_identity(nc, ident)
        lxt = singles.tile([P, CHUNKS, PAIRS], F32)
        for c in range(CHUNKS):
            pt = psum.tile([P, PAIRS], F32)
            nc.tensor.transpose(pt, lxm[:, c * P:(c + 1) * P], ident)
            nc.vector.tensor_copy(out=lxt[:, c, :], in_=pt)
        ext = singles.tile([P, CHUNKS, PAIRS], F32)
        nc.scalar.activation(out=ext, in_=lxt, func=AF.Exp,
                             bias=0.0, scale=1.0)

        # ---- y-side coefficients on all 128 partitions -------------------
        lab128 = singles.tile([P, BPC * 2 * NCLS], F32)
        lsrc = labels[:, :].rearrange("b t -> (b t)")
        nc.gpsimd.dma_start(
            out=lab128,
            in_=bass.AP(tensor=lsrc.tensor, offset=lsrc.offset,
                        ap=[[0, P], [1, BPC * 2 * NCLS]]),
        )
        lw128 = singles.tile([P, 1], F32)
        nc.gpsimd.dma_start(out=lw128, in_=logw[:, :].to_broadcast((P, 1)))

        # nmy128[:, p] = -my_p on every partition     (DVE)
        nmy128 = singles.tile([P, PAIRS], F32)
        nc.vector.tensor_scalar_mul(
            out=nmy128,
            in0=lab128[:, :].rearrange("p (q two) -> p q two", two=2)[:, :, 1],
            scalar1=-float(H))
        # sc_y = -exp(-lw-lnH)/40 and 2*sc_y          (ACT back-to-back)
        t128 = singles.tile([P, 1], F32)
        nc.vector.tensor_scalar(out=t128, in0=lw128, scalar1=-1.0,
                                scalar2=-LN_H, op0=mybir.AluOpType.mult,
                                op1=mybir.AluOpType.add)
        inv128 = singles.tile([P, 1], F32)
        nc.scalar.activation(out=inv128, in_=t128, func=AF.Exp,
                             bias=0.0, scale=1.0)
        scy128 = singles.tile([P, 1], F32)
        nc.vector.tensor_scalar_mul(out=scy128, in0=inv128, scalar1=-0.025)
        scy2 = singles.tile([P, 1], F32)
        nc.vector.tensor_scalar_mul(out=scy2, in0=scy128, scalar1=2.0)

        # a_p = 2*sc_y*(-my_p);  c_p = sc_y*my_p^2    (DVE)
        a128 = singles.tile([P, PAIRS], F32)
        nc.vector.tensor_scalar_mul(out=a128, in0=nmy128, scalar1=scy2)
        m2 = singles.tile([P, PAIRS], F32)
        nc.vector.tensor_mul(out=m2, in0=nmy128, in1=nmy128)
        c128 = singles.tile([P, PAIRS], F32)
        nc.vector.tensor_scalar_mul(out=c128, in0=m2, scalar1=scy128)

        # U(w) = exp(sc_y * w^2), shared by all DVE-path pairs  (ACT)
        w2 = singles.tile([P, W], F32)
        nc.scalar.activation(out=w2, in_=iog, func=AF.Square,
                             bias=0.0, scale=1.0)
        ubuf = singles.tile([P, W], F32)
        nc.scalar.activation(out=ubuf, in_=w2, func=AF.Exp,
                             bias=0.0, scale=scy128)

        # ---- main loop ---------------------------------------------------
        for p in range(PAIRS):
            st = stage.tile([P, CHUNKS, W], F32)
            if p in ACT_PAIRS:
                # all-ACT path: sq_y then a final Exp per chunk
                sq = ybuf.tile([P, W], F32, tag="sq")
                nc.scalar.activation(out=sq, in_=iog, func=AF.Square,
                                     bias=nmy128[:, p:p + 1], scale=1.0)
                for c in range(CHUNKS):
                    nc.scalar.activation(out=st[:, c, :], in_=sq,
                                         func=AF.Exp,
                                         bias=lxt[:, c, p:p + 1],
                                         scale=-1.0)
            else:
                # DVE path: E_p on ACT, U*E_p and scalar mults on DVE
                ep = ybuf.tile([P, W], F32, tag="ep")
                nc.scalar.activation(out=ep, in_=iog, func=AF.Exp,
                                     bias=c128[:, p:p + 1],
                                     scale=a128[:, p:p + 1])
                eyb = ybuf.tile([P, W], F32, tag="eyb")
                nc.vector.tensor_mul(out=eyb, in0=ubuf, in1=ep)
                for c in range(CHUNKS):
                    scal = ext[:, c, p:p + 1]
                    if MULT_ENGINE[p * CHUNKS + c] == "v":
                        nc.vector.tensor_scalar_mul(out=st[:, c, :],
                                                    in0=eyb, scalar1=scal)
                    else:
                        nc.scalar.mul(out=st[:, c, :], in_=eyb, mul=scal)
            # rows of pair p are h = c*128 + par ; DRAM side iterates
            # (par, c, w) to match the SBUF tile layout.
            nc.sync.dma_start(
                out=out[p * H:(p + 1) * H, :].rearrange(
                    "(c par) w -> par c w", par=P),
                in_=st,
            )
    nc.finalize()
    return nc


LAST_RESULTS = None  # BassKernelResults of the most recent kernel() call


def kernel(x: np.ndarray, labels: np.ndarray,
           log_weight: np.ndarray, **run_kwargs) -> np.ndarray:
    global LAST_RESULTS
    del x  # only its (hardcoded) shape matters
    nc = build_bass()
    labels = np.ascontiguousarray(labels, dtype=np.float32)
    lw = np.ascontiguousarray(log_weight, dtype=np.float32).reshape(1, 1)
    in_maps = [
        {"labels": labels[i * BPC:(i + 1) * BPC], "log_weight": lw}
        for i in range(N_CORES)
    ]
    res = run_bass_kernel_spmd(nc, in_maps, core_ids=list(range(N_CORES)),
                               **run_kwargs)
    LAST_RESULTS = res
    outs = [r["out"].reshape(BPC, NCLS, H, W) for r in res.results]
    return np.concatenate(outs, axis=0)


if __name__ == "__main__":
    rng = np.random.default_rng(0)
    x = rng.standard_normal((B, CH, H, W), dtype=np.float32)
    labels = rng.random((B, 2 * NCLS), dtype=np.float32)
    lw = rng.random((1, 1, 1, 1), dtype=np.float32)
    y = kernel(x=x, labels=labels, log_weight=lw)
    print(y.shape, y.dtype, y.min(), y.max())



# revision 6
# speedup vs baseline: 1.1081x; 1.1081x over previous
"""Trainium2 Bass kernel for nn_HeatmapLayer: separable Gaussian heatmaps.

Reference math (per batch b, class c):
    mx = labels[b, 2c] * H ; my = labels[b, 2c+1] * W          (H = W = 384)
    sigma = H * exp(log_weight)
    dx2[h] = (h - mx)^2 / sigma        ; normalized by its min over h
    dy2[w] = (w - my)^2 / (20 * sigma) ; normalized by its min over w
    out[b,c,h,w] = exp(-0.5*(dx2[h] + dy2[w])) = ex[h] * ey[w]

Each (b,c) heatmap is a rank-1 outer product of two 384-length
profiles.  Per core (pure data parallel over batch): 2 batches -> 12
(b,c) pairs.

v2 design (PE outer-product): profiles ex/ey are computed once on 12
partitions ([12, 384], ~same cost as a single [128, 384] op).  The
idle PE engine broadcasts ey to all 128 partitions (one K=12 matmul
per pair against a 0/1 block-selector, float32r at full speed), and
the per-(pair,chunk) finals are single DVE/ACT ops reading PSUM and
scaling by the transposed x-profile EXT (per-partition scalar).  Each
output element is written by exactly ONE DVE/ACT op -- the engine
minimum -- so the 16 DMA engines (~19.7us aggregate for 7.08MB/core)
become the bottleneck instead of compute.

Output rows are staged as h = 3*par + c so each SBUF partition maps
to one contiguous 4608B DRAM run (bigger DMA descriptors than the
c*128+par layout).  EXT is built by PE-transposing stride-3 slices of
ex, giving ext[par, c, p] = ex_p[3*par + c].
"""

import numpy as np
from contextlib import ExitStack

import concourse.bacc as bacc
import concourse.bass as bass
import concourse.tile as tile
from concourse import mybir
from concourse.bass_utils import run_bass_kernel_spmd
from concourse.masks import make_identity

B, CH, H, W = 16, 3, 384, 384
NCLS = 6
N_CORES = 8
BPC = B // N_CORES            # batches per core = 2
PAIRS = BPC * NCLS            # (b,c) pairs per core = 12
P = 128
C3 = H // P                   # 3 chunks of 128 rows
LN_H = float(np.log(H))
F32 = mybir.dt.float32
BF16 = mybir.dt.bfloat16
AF = mybir.ActivationFunctionType

# engine for the 36 finals, by flat index (p*3+c): alternate per pair
# so DVE and ACT each take 18.
BALANCE = "".join("vav" if p % 2 == 0 else "ava" for p in range(PAIRS))


def build_bass() -> bass.Bass:
    nc = bacc.Bacc("TRN2", target_bir_lowering=False, debug=False,
                   num_devices=N_CORES)
    labels = nc.dram_tensor("labels", [BPC, 2 * NCLS], F32,
                            kind="ExternalInput")
    logw = nc.dram_tensor("log_weight", [1, 1], F32, kind="ExternalInput")
    out = nc.dram_tensor("out", [PAIRS * H, W], F32, kind="ExternalOutput")

    with ExitStack() as ctx:
        tc = ctx.enter_context(tile.TileContext(nc))
        singles = ctx.enter_context(tc.tile_pool(name="singles", bufs=1))
        psT = ctx.enter_context(tc.tile_pool(name="psT", bufs=2,
                                             space="PSUM"))
        psB = ctx.enter_context(tc.tile_pool(name="psB", bufs=6,
                                             space="PSUM"))
        stage = ctx.enter_context(tc.tile_pool(name="stage", bufs=4))

        # ---- inputs first: their DMA latency dominates startup -----------
        lab = singles.tile([PAIRS, 2], F32)     # (mx_raw, my_raw) in [0,1)
        nc.sync.dma_start(
            out=lab,
            in_=labels[:, :].rearrange("b (q two) -> (b q) two", two=2),
        )
        lwb = singles.tile([PAIRS, 1], F32)
        nc.gpsimd.dma_start(out=lwb, in_=logw[:, :].to_broadcast((PAIRS, 1)))

        # ---- constants on gpsimd (no input deps) -------------------------
        iog = singles.tile([PAIRS, W], F32)     # 0..383 on 12 partitions
        nc.gpsimd.iota(iog, pattern=[[1, W]], base=0, channel_multiplier=0,
                       allow_small_or_imprecise_dtypes=True)
        ident = singles.tile([PAIRS, PAIRS], F32)
        make_identity(nc, ident)
        # bigsel[k, p*128 + m] = 1.0 if k == p else 0.0   (block selector)
        bigsel = singles.tile([PAIRS, PAIRS * P], BF16)
        nc.gpsimd.memset(bigsel, 0.0)
        nc.gpsimd.affine_select(
            out=bigsel, in_=bigsel,
            compare_op=mybir.AluOpType.not_equal, fill=1.0, base=0,
            pattern=[[-1, PAIRS], [0, P]], channel_multiplier=1,
        )

        # ---- per-pair scalars ---------------------------------------------
        # sq = (lab - h/384)^2 = (h - m)^2 / 384^2  -> scales fold into sc2.
        # inv_s = exp(-lw - lnH);  x scale: -384^2*inv_s/2, y: -384^2*inv_s/40
        nlw = singles.tile([PAIRS, 1], F32)
        nc.vector.tensor_scalar(out=nlw, in0=lwb, scalar1=-1.0,
                                scalar2=-LN_H, op0=mybir.AluOpType.mult,
                                op1=mybir.AluOpType.add)
        inv_s = singles.tile([PAIRS, 1], F32)
        nc.scalar.activation(out=inv_s, in_=nlw, func=AF.Exp,
                             bias=0.0, scale=1.0)
        sc2 = singles.tile([PAIRS, 2], F32)
        nc.vector.tensor_scalar_mul(out=sc2[:, 0:1], in0=inv_s,
                                    scalar1=-0.5 * float(H) * float(H))
        nc.vector.tensor_scalar_mul(out=sc2[:, 1:2], in0=inv_s,
                                    scalar1=-0.025 * float(H) * float(H))

        # ---- squared-distance profiles, one min-reduce --------------------
        sqxy = singles.tile([PAIRS, 2, W], F32)
        nc.scalar.activation(out=sqxy[:, 0, :], in_=iog, func=AF.Square,
                             bias=lab[:, 0:1], scale=-1.0 / float(H))
        nc.scalar.activation(out=sqxy[:, 1, :], in_=iog, func=AF.Square,
                             bias=lab[:, 1:2], scale=-1.0 / float(H))
        mn2 = singles.tile([PAIRS, 2], F32)
        nc.vector.tensor_reduce(out=mn2, in_=sqxy, axis=mybir.AxisListType.X,
                                op=mybir.AluOpType.min)
        # nb = -sc2 * mn2  (bias so that exp arg = sc2*(sq - mn) <= 0)
        bb = singles.tile([PAIRS, 2], F32)
        nc.vector.tensor_mul(out=bb, in0=mn2, in1=sc2)
        nb = singles.tile([PAIRS, 2], F32)
        nc.vector.tensor_scalar_mul(out=nb, in0=bb, scalar1=-1.0)

        # ---- linear-domain profiles ---------------------------------------
        ey = singles.tile([PAIRS, W], BF16)     # y profile (matmul rhs)
        nc.scalar.activation(out=ey, in_=sqxy[:, 1, :], func=AF.Exp,
                             bias=nb[:, 1:2], scale=sc2[:, 1:2])
        ex = singles.tile([PAIRS, W], F32)      # x profile (to transpose)
        nc.scalar.activation(out=ex, in_=sqxy[:, 0, :], func=AF.Exp,
                             bias=nb[:, 0:1], scale=sc2[:, 0:1])

        # ---- EXT[par, c, p] = ex_p[3*par + c] via 3 strided PE transposes -
        exr = ex[:, :].rearrange("p (h c) -> p c h", c=C3)
        ext = singles.tile([P, C3, PAIRS], F32)
        for c in range(C3):
            pt = psT.tile([P, PAIRS], F32)
            nc.tensor.transpose(pt, exr[:, c, :], ident)
            nc.vector.tensor_copy(out=ext[:, c, :], in_=pt)

        # ---- main loop: PE broadcast + one final op per output chunk ------
        for p in range(PAIRS):
            ps = psB.tile([P, W], F32)
            nc.tensor.matmul(ps, bigsel[:, p * P:(p + 1) * P],
                             ey, start=True, stop=True)
            st = stage.tile([P, C3, W], F32)
            for c in range(C3):
                scal = ext[:, c, p:p + 1]
                if BALANCE[p * C3 + c] == "v":
                    nc.vector.tensor_scalar_mul(out=st[:, c, :], in0=ps,
                                                scalar1=scal)
                else:
                    nc.scalar.mul(out=st[:, c, :], in_=ps, mul=scal)
            # DRAM row (within pair p) = 3*par + c: one contiguous 4608B
            # run per partition.
            nc.sync.dma_start(
                out=out[p * H:(p + 1) * H, :].rearrange(
                    "(par c) w -> par c w", c=C3),
                in_=st,
            )
    nc.finalize()
    return nc


LAST_RESULTS = None  # BassKernelResults of the most recent kernel() call


def kernel(x: np.ndarray, labels: np.ndarray,
           log_weight: np.ndarray, **run_kwargs) -> np.ndarray:
    global LAST_RESULTS
    del x  # only its (hardcoded) shape matters
    nc = build_bass()
    labels = np.ascontiguousarray(labels, dtype=np.float32)
    lw = np.ascontiguousarray(log_weight, dtype=np.float32).reshape(1, 1)
    in_maps = [
        {"labels": labels[i * BPC:(i + 1) * BPC], "log_weight": lw}
        for i in range(N_CORES)
    ]
    res = run_bass_kernel_spmd(nc, in_maps, core_ids=list(range(N_CORES)),
                               **run_kwargs)
    LAST_RESULTS = res
    outs = [r["out"].reshape(BPC, NCLS, H, W) for r in res.results]
    return np.concatenate(outs, axis=0)


if __name__ == "__main__":
    rng = np.random.default_rng(0)
    x = rng.standard_normal((B, CH, H, W), dtype=np.float32)
    labels = rng.random((B, 2 * NCLS), dtype=np.float32)
    lw = rng.random((1, 1, 1, 1), dtype=np.float32)
    y = kernel(x=x, labels=labels, log_weight=lw)
    print(y.shape, y.dtype, y.min(), y.max())


# revision 7
# speedup vs baseline: 1.2508x; 1.1288x over previous
"""Trainium2 Bass kernel for nn_HeatmapLayer: separable Gaussian heatmaps.

Reference math (per batch b, class c):
    mx = labels[b, 2c] * H ; my = labels[b, 2c+1] * W          (H = W = 384)
    sigma = H * exp(log_weight)
    dx2[h] = (h - mx)^2 / sigma        ; normalized by its min over h
    dy2[w] = (w - my)^2 / (20 * sigma) ; normalized by its min over w
    out[b,c,h,w] = exp(-0.5*(dx2[h] + dy2[w])) = ex[h] * ey[w]

Each (b,c) heatmap is a rank-1 outer product of two 384-length
profiles.  Per core (pure data parallel over batch): 2 batches -> 12
(b,c) pairs.

v3 design:
  * PE outer-product: profiles ex/ey are computed once on 12
    partitions.  The idle PE engine broadcasts ey (bf16) to all 128
    partitions (one K=12 matmul per pair against a 0/1 block-selector)
    and the per-(pair,chunk) finals are single DVE/ACT ops reading
    PSUM and scaling by the transposed x-profile EXT (per-partition
    scalar).  Each output element is written by exactly ONE DVE/ACT
    op, so the 16 DMA engines (~19.7us aggregate for 7.08MB/core) are
    the floor.
  * All constants (grid, identity, block-selector) and the replicated
    log_weight arrive as host-provided DMA inputs: the gpsimd engine
    (slow ~6us wake + slow iota/affine_select) is not used at all.
  * The per-axis min of (h-m)^2 is computed EXACTLY from the labels
    alone with the +-2^23 round-to-integer trick + clamp (tiny DVE
    ops, hidden under the ACT Squares) instead of a 384-wide reduce.
  * Output rows are staged as h = 3*par + c so each SBUF partition is
    one contiguous 4608B DRAM run. EXT[par,c,p] = ex_p[3*par+c] comes
    from 3 stride-3 PE transposes.
"""

import numpy as np
from contextlib import ExitStack

import concourse.bacc as bacc
import concourse.bass as bass
import concourse.tile as tile
from concourse import mybir
from concourse.bass_utils import run_bass_kernel_spmd

B, CH, H, W = 16, 3, 384, 384
NCLS = 6
N_CORES = 8
BPC = B // N_CORES            # batches per core = 2
PAIRS = BPC * NCLS            # (b,c) pairs per core = 12
P = 128
C3 = H // P                   # 3 chunks of 128 rows
LN_H = float(np.log(H))
RND = 12582912.0              # 1.5 * 2^23: adding+subtracting rounds to int
F32 = mybir.dt.float32
BF16 = mybir.dt.bfloat16
AF = mybir.ActivationFunctionType
ALU = mybir.AluOpType

# engine for the 36 finals, by flat index (p*3+c): DVE 20 / ACT 16.
BALANCE = ("vav" + "vva") * 6


def build_bass() -> bass.Bass:
    nc = bacc.Bacc("TRN2", target_bir_lowering=False, debug=False,
                   num_devices=N_CORES)
    labels = nc.dram_tensor("labels", [PAIRS, 2], F32, kind="ExternalInput")
    logw12 = nc.dram_tensor("logw12", [PAIRS, 1], F32, kind="ExternalInput")
    grid12 = nc.dram_tensor("grid12", [PAIRS, W], F32, kind="ExternalInput")
    identd = nc.dram_tensor("identd", [PAIRS, PAIRS], F32,
                            kind="ExternalInput")
    bigsld = nc.dram_tensor("bigsld", [PAIRS, PAIRS * P], BF16,
                            kind="ExternalInput")
    out = nc.dram_tensor("out", [PAIRS * H, W], F32, kind="ExternalOutput")

    with ExitStack() as ctx:
        tc = ctx.enter_context(tile.TileContext(nc))
        singles = ctx.enter_context(tc.tile_pool(name="singles", bufs=1))
        psT = ctx.enter_context(tc.tile_pool(name="psT", bufs=2,
                                             space="PSUM"))
        psB = ctx.enter_context(tc.tile_pool(name="psB", bufs=6,
                                             space="PSUM"))
        stage = ctx.enter_context(tc.tile_pool(name="stage", bufs=12))

        # ---- input DMAs split over both HWDGE queues ----------------------
        lab = singles.tile([PAIRS, 2], F32)     # (mx, my)/H in [0,1)
        nc.sync.dma_start(out=lab, in_=labels[:, :])
        iog = singles.tile([PAIRS, W], F32)     # j/384 grid on 12 partitions
        nc.sync.dma_start(out=iog, in_=grid12[:, :])
        lwb = singles.tile([PAIRS, 1], F32)
        nc.scalar.dma_start(out=lwb, in_=logw12[:, :])
        ident = singles.tile([PAIRS, PAIRS], F32)
        nc.scalar.dma_start(out=ident, in_=identd[:, :])
        bigsel = singles.tile([PAIRS, PAIRS * P], BF16)
        nc.scalar.dma_start(out=bigsel, in_=bigsld[:, :])

        # ---- per-pair scalars (DVE, tiny; hidden under ACT Squares) -------
        # inv_s = exp(-lw - lnH); grid is j/384 so sq=(lab-j/384)^2 and the
        # 384^2 folds into the exp scales.
        nlw = singles.tile([PAIRS, 1], F32)
        nc.vector.tensor_scalar(out=nlw, in0=lwb, scalar1=-1.0,
                                scalar2=-LN_H, op0=ALU.mult, op1=ALU.add)
        inv_s = singles.tile([PAIRS, 1], F32)
        nc.scalar.activation(out=inv_s, in_=nlw, func=AF.Exp,
                             bias=0.0, scale=1.0)
        sc2 = singles.tile([PAIRS, 2], F32)     # exp scale (negative)
        nc.vector.tensor_scalar_mul(out=sc2[:, 0:1], in0=inv_s,
                                    scalar1=-0.5 * float(H) * float(H))
        nc.vector.tensor_scalar_mul(out=sc2[:, 1:2], in0=inv_s,
                                    scalar1=-0.025 * float(H) * float(H))
        posc2 = singles.tile([PAIRS, 2], F32)   # |sc2| / 384^2
        nc.vector.tensor_scalar_mul(out=posc2[:, 0:1], in0=inv_s,
                                    scalar1=0.5)
        nc.vector.tensor_scalar_mul(out=posc2[:, 1:2], in0=inv_s,
                                    scalar1=0.025)

        # exact min of (h-m)^2 over integer h in [0,383], from labels only:
        # h* = clamp(round(m), max 383), min = (m-h*)^2  (in grid^2 units)
        m2c = singles.tile([PAIRS, 2], F32)     # m = 384*lab
        nc.vector.tensor_scalar_mul(out=m2c, in0=lab, scalar1=float(H))
        t1 = singles.tile([PAIRS, 2], F32)
        nc.vector.tensor_scalar_add(out=t1, in0=m2c, scalar1=RND)
        rr = singles.tile([PAIRS, 2], F32)      # round(m) (half-to-even)
        nc.vector.tensor_scalar_add(out=rr, in0=t1, scalar1=-RND)
        rc = singles.tile([PAIRS, 2], F32)      # clamp to grid max
        nc.vector.tensor_scalar_min(out=rc, in0=rr, scalar1=float(H - 1))
        dd = singles.tile([PAIRS, 2], F32)
        nc.vector.tensor_sub(out=dd, in0=m2c, in1=rc)
        mn = singles.tile([PAIRS, 2], F32)
        nc.vector.tensor_mul(out=mn, in0=dd, in1=dd)
        nb = singles.tile([PAIRS, 2], F32)      # exp bias: -sc2*min >= 0
        nc.vector.tensor_mul(out=nb, in0=mn, in1=posc2)

        # ---- profiles: y first (feeds the matmuls) ------------------------
        sqxy = singles.tile([PAIRS, 2, W], F32)
        nc.scalar.activation(out=sqxy[:, 1, :], in_=iog, func=AF.Square,
                             bias=lab[:, 1:2], scale=-1.0)
        ey = singles.tile([PAIRS, W], BF16)     # y profile (matmul rhs)
        nc.scalar.activation(out=ey, in_=sqxy[:, 1, :], func=AF.Exp,
                             bias=nb[:, 1:2], scale=sc2[:, 1:2])
        nc.scalar.activation(out=sqxy[:, 0, :], in_=iog, func=AF.Square,
                             bias=lab[:, 0:1], scale=-1.0)
        ex = singles.tile([PAIRS, W], F32)      # x profile (to transpose)
        nc.scalar.activation(out=ex, in_=sqxy[:, 0, :], func=AF.Exp,
                             bias=nb[:, 0:1], scale=sc2[:, 0:1])

        # ---- first matmul can go as soon as ey lands ----------------------
        def pair_matmul(p):
            ps = psB.tile([P, W], F32)
            nc.tensor.matmul(ps, bigsel[:, p * P:(p + 1) * P], ey,
                             start=True, stop=True)
            return ps

        ps0 = pair_matmul(0)

        # ---- EXT[par, c, p] = ex_p[3*par + c] via 3 strided PE transposes -
        exr = ex[:, :].rearrange("p (h c) -> p c h", c=C3)
        ext = singles.tile([P, C3, PAIRS], F32)
        for c in range(C3):
            pt = psT.tile([P, PAIRS], F32)
            nc.tensor.transpose(pt, exr[:, c, :], ident)
            nc.vector.tensor_copy(out=ext[:, c, :], in_=pt)

        # ---- main loop: one final op per output chunk, then one DMA -------
        for p in range(PAIRS):
            ps = ps0 if p == 0 else pair_matmul(p)
            st = stage.tile([P, C3, W], F32)
            for c in range(C3):
                scal = ext[:, c, p:p + 1]
                if BALANCE[p * C3 + c] == "v":
                    nc.vector.tensor_scalar_mul(out=st[:, c, :], in0=ps,
                                                scalar1=scal)
                else:
                    nc.scalar.mul(out=st[:, c, :], in_=ps, mul=scal)
            # DRAM row (within pair p) = 3*par + c: one contiguous 4608B
            # run per partition.
            nc.sync.dma_start(
                out=out[p * H:(p + 1) * H, :].rearrange(
                    "(par c) w -> par c w", c=C3),
                in_=st,
            )
    nc.finalize()
    return nc


LAST_RESULTS = None  # BassKernelResults of the most recent kernel() call


def _host_consts():
    import ml_dtypes
    grid = np.tile((np.arange(W, dtype=np.float32) / np.float32(W)),
                   (PAIRS, 1)).astype(np.float32)
    ident = np.eye(PAIRS, dtype=np.float32)
    bigsel = np.kron(np.eye(PAIRS, dtype=np.float32),
                     np.ones((1, P), dtype=np.float32)
                     ).astype(ml_dtypes.bfloat16)
    return grid, ident, bigsel


def kernel(x: np.ndarray, labels: np.ndarray,
           log_weight: np.ndarray, **run_kwargs) -> np.ndarray:
    global LAST_RESULTS
    del x  # only its (hardcoded) shape matters
    nc = build_bass()
    labels = np.ascontiguousarray(labels, dtype=np.float32)
    lw12 = np.tile(np.float32(log_weight).reshape(1, 1), (PAIRS, 1))
    grid, ident, bigsel = _host_consts()
    in_maps = [
        {
            "labels": labels[i * BPC:(i + 1) * BPC].reshape(PAIRS, 2),
            "logw12": lw12, "grid12": grid, "identd": ident,
            "bigsld": bigsel,
        }
        for i in range(N_CORES)
    ]
    res = run_bass_kernel_spmd(nc, in_maps, core_ids=list(range(N_CORES)),
                               **run_kwargs)
    LAST_RESULTS = res
    outs = [r["out"].reshape(BPC, NCLS, H, W) for r in res.results]
    return np.concatenate(outs, axis=0)


if __name__ == "__main__":
    rng = np.random.default_rng(0)
    x = rng.standard_normal((B, CH, H, W), dtype=np.float32)
    labels = rng.random((B, 2 * NCLS), dtype=np.float32)
    lw = rng.random((1, 1, 1, 1), dtype=np.float32)
    y = kernel(x=x, labels=labels, log_weight=lw)
    print(y.shape, y.dtype, y.min(), y.max())


# revision 8
# speedup vs baseline: 1.2599x; 1.0073x over previous
"""Trainium2 Bass kernel for nn_HeatmapLayer: separable Gaussian heatmaps.

Reference math (per batch b, class c):
    mx = labels[b, 2c] * H ; my = labels[b, 2c+1] * W          (H = W = 384)
    sigma = H * exp(log_weight)
    dx2[h] = (h - mx)^2 / sigma        ; normalized by its min over h
    dy2[w] = (w - my)^2 / (20 * sigma) ; normalized by its min over w
    out[b,c,h,w] = exp(-0.5*(dx2[h] + dy2[w])) = ex[h] * ey[w]

Each (b,c) heatmap is a rank-1 outer product of two 384-length
profiles; 2 batches x 6 classes = 12 pairs per core (batch-parallel
over 8 cores).  The kernel is output-DMA-bound: 7.08MB/core over 16
DMA engines at ~360GB/s aggregate = ~19.7us, so everything else is
organized to start that drain as early as possible and keep it
gap-free.

Key structure:
  * PE outer-product: ex/ey profiles are computed once on 12
    partitions; the idle PE engine broadcasts ey (bf16) to all 128
    partitions (one K=12 matmul per pair against a 0/1 block-selector)
    and each output chunk is produced by exactly ONE DVE/ACT op
    reading PSUM and scaling by the transposed x-profile EXT
    (per-partition scalar) -- the engine minimum per output element.
  * The start-of-kernel all-engine barrier is rebuilt WITHOUT the
    GpSimd/Pool engine: its Q7 cores take ~6us to boot and nothing in
    this kernel uses them (constants arrive as host-provided DMA
    inputs; no float activation biases -> no const-AP reads, which are
    the only thing the stock barrier protects here).
  * ACT's Exp table (~1.3us load) is warmed by a dummy Exp on a
    memset tile before the inputs even arrive.
  * All f32 inputs ride in ONE packed [12, 399] DMA (labels, logw,
    grid, identity); the bf16 block-selector is a second DMA on the
    ACT HWDGE queue.
  * The per-axis min of (h-m)^2 is computed EXACTLY from the labels
    alone with the +-2^23 round-to-integer trick + clamp (tiny DVE
    ops hidden under the ACT Squares) instead of a 384-wide reduce.
  * Output rows are staged as h = 3*par + c so each SBUF partition is
    one contiguous 4608B DRAM run.  EXT[par,c,p] = ex_p[3*par+c] comes
    from 3 stride-3 PE transposes.
"""

import numpy as np
from contextlib import ExitStack

import concourse.bacc as bacc
import concourse.bass as bass
import concourse.tile as tile
from concourse import mybir
from concourse.bass_utils import run_bass_kernel_spmd

B, CH, H, W = 16, 3, 384, 384
NCLS = 6
N_CORES = 8
BPC = B // N_CORES            # batches per core = 2
PAIRS = BPC * NCLS            # (b,c) pairs per core = 12
P = 128
C3 = H // P                   # 3 chunks of 128 rows
LN_H = float(np.log(H))
RND = 12582912.0              # 1.5 * 2^23: add+subtract rounds to integer
F32 = mybir.dt.float32
BF16 = mybir.dt.bfloat16
AF = mybir.ActivationFunctionType
ALU = mybir.AluOpType

# packed f32 input layout: [labels(2) | logw(1) | grid(384) | ident(12)]
PK_LAB, PK_LW, PK_GRID, PK_ID = 0, 2, 3, 387
PK_N = PK_ID + PAIRS

# engine for the 36 finals, by flat index (p*3+c): DVE 18 / ACT 18.
BALANCE = ("vav" + "ava") * 6


def _barrier_without_pool(self, *, sem_only: bool = False):
    engines = [e for e in self.engines if e != mybir.EngineType.Pool]
    if sem_only:
        for inst in self._sem_only_all_engine_barrier_insts("aeb"):
            self.engines[inst.engine].add_instruction(inst)
    else:
        self.multi_engine_barrier(engines)


def build_bass() -> bass.Bass:
    orig_barrier = bass.Bass.all_engine_barrier
    bass.Bass.all_engine_barrier = _barrier_without_pool
    try:
        nc = bacc.Bacc("TRN2", target_bir_lowering=False, debug=False,
                       num_devices=N_CORES)
        _build_body(nc)
        nc.finalize()
    finally:
        bass.Bass.all_engine_barrier = orig_barrier
    return nc


def _build_body(nc) -> None:
    pack1 = nc.dram_tensor("pack1", [PAIRS, PK_N], F32, kind="ExternalInput")
    bigsld = nc.dram_tensor("bigsld", [PAIRS, PAIRS * P], BF16,
                            kind="ExternalInput")
    out = nc.dram_tensor("out", [PAIRS * H, W], F32, kind="ExternalOutput")

    with ExitStack() as ctx:
        tc = ctx.enter_context(tile.TileContext(nc))
        singles = ctx.enter_context(tc.tile_pool(name="singles", bufs=1))
        psT = ctx.enter_context(tc.tile_pool(name="psT", bufs=2,
                                             space="PSUM"))
        psB = ctx.enter_context(tc.tile_pool(name="psB", bufs=6,
                                             space="PSUM"))
        stage = ctx.enter_context(tc.tile_pool(name="stage", bufs=12))

        # ---- input DMAs, one per HWDGE queue ------------------------------
        pk = singles.tile([PAIRS, PK_N], F32)
        nc.sync.dma_start(out=pk, in_=pack1[:, :])
        bigsel = singles.tile([PAIRS, PAIRS * P], BF16)
        nc.scalar.dma_start(out=bigsel, in_=bigsld[:, :])

        lab = pk[:, PK_LAB:PK_LAB + 2]          # (mx, my)/H in [0,1)
        lwb = pk[:, PK_LW:PK_LW + 1]
        iog = pk[:, PK_GRID:PK_GRID + W]        # j/384 grid
        ident = pk[:, PK_ID:PK_ID + PAIRS]

        # ---- warm the ACT Exp table before inputs arrive ------------------
        zz = singles.tile([PAIRS, 2], F32)
        nc.vector.memset(zz, 0.0)
        warm = singles.tile([PAIRS, 1], F32)
        nc.scalar.activation(out=warm, in_=zz[:, 0:1], func=AF.Exp,
                             bias=zz[:, 1:2], scale=1.0)
        zcol = zz[:, 1:2]                        # zeros bias AP

        # ---- per-pair scalars (DVE, tiny; hidden under ACT Squares) -------
        # inv_s = exp(-lw - lnH); grid is j/384 so sq=(lab-j/384)^2 and the
        # 384^2 folds into the exp scales.
        nlw = singles.tile([PAIRS, 1], F32)
        nc.vector.tensor_scalar(out=nlw, in0=lwb, scalar1=-1.0,
                                scalar2=-LN_H, op0=ALU.mult, op1=ALU.add)
        inv_s = singles.tile([PAIRS, 1], F32)
        nc.scalar.activation(out=inv_s, in_=nlw, func=AF.Exp,
                             bias=zcol, scale=1.0)
        sc2 = singles.tile([PAIRS, 2], F32)     # exp scale (negative)
        nc.vector.tensor_scalar_mul(out=sc2[:, 0:1], in0=inv_s,
                                    scalar1=-0.5 * float(H) * float(H))
        nc.vector.tensor_scalar_mul(out=sc2[:, 1:2], in0=inv_s,
                                    scalar1=-0.025 * float(H) * float(H))
        posc2 = singles.tile([PAIRS, 2], F32)   # |sc2| / 384^2
        nc.vector.tensor_scalar_mul(out=posc2[:, 0:1], in0=inv_s,
                                    scalar1=0.5)
        nc.vector.tensor_scalar_mul(out=posc2[:, 1:2], in0=inv_s,
                                    scalar1=0.025)

        # exact min of (h-m)^2 over integer h in [0,383], from labels only:
        # h* = clamp(round(m), max 383), min = (m-h*)^2  (in grid^2 units)
        m2c = singles.tile([PAIRS, 2], F32)     # m = 384*lab
        nc.vector.tensor_scalar_mul(out=m2c, in0=lab, scalar1=float(H))
        t1 = singles.tile([PAIRS, 2], F32)
        nc.vector.tensor_scalar_add(out=t1, in0=m2c, scalar1=RND)
        rr = singles.tile([PAIRS, 2], F32)      # round(m) (half-to-even)
        nc.vector.tensor_scalar_add(out=rr, in0=t1, scalar1=-RND)
        rc = singles.tile([PAIRS, 2], F32)      # clamp to grid max
        nc.vector.tensor_scalar_min(out=rc, in0=rr, scalar1=float(H - 1))
        dd = singles.tile([PAIRS, 2], F32)
        nc.vector.tensor_sub(out=dd, in0=m2c, in1=rc)
        mn = singles.tile([PAIRS, 2], F32)
        nc.vector.tensor_mul(out=mn, in0=dd, in1=dd)
        nb = singles.tile([PAIRS, 2], F32)      # exp bias: -sc2*min >= 0
        nc.vector.tensor_mul(out=nb, in0=mn, in1=posc2)

        # ---- profiles: y first (feeds the matmuls) ------------------------
        sqxy = singles.tile([PAIRS, 2, W], F32)
        nc.scalar.activation(out=sqxy[:, 1, :], in_=iog, func=AF.Square,
                             bias=lab[:, 1:2], scale=-1.0)
        ey = singles.tile([PAIRS, W], BF16)     # y profile (matmul rhs)
        nc.scalar.activation(out=ey, in_=sqxy[:, 1, :], func=AF.Exp,
                             bias=nb[:, 1:2], scale=sc2[:, 1:2])
        nc.scalar.activation(out=sqxy[:, 0, :], in_=iog, func=AF.Square,
                             bias=lab[:, 0:1], scale=-1.0)
        ex = singles.tile([PAIRS, W], F32)      # x profile (to transpose)
        nc.scalar.activation(out=ex, in_=sqxy[:, 0, :], func=AF.Exp,
                             bias=nb[:, 0:1], scale=sc2[:, 0:1])

        # ---- first matmul can go as soon as ey lands ----------------------
        def pair_matmul(p):
            ps = psB.tile([P, W], F32)
            nc.tensor.matmul(ps, bigsel[:, p * P:(p + 1) * P], ey,
                             start=True, stop=True)
            return ps

        ps0 = pair_matmul(0)

        # ---- EXT[par, c, p] = ex_p[3*par + c] via 3 strided PE transposes -
        exr = ex[:, :].rearrange("p (h c) -> p c h", c=C3)
        ext = singles.tile([P, C3, PAIRS], F32)
        for c in range(C3):
            pt = psT.tile([P, PAIRS], F32)
            nc.tensor.transpose(pt, exr[:, c, :], ident)
            nc.vector.tensor_copy(out=ext[:, c, :], in_=pt)

        # ---- main loop: one final op per output chunk, then one DMA -------
        for p in range(PAIRS):
            ps = ps0 if p == 0 else pair_matmul(p)
            st = stage.tile([P, C3, W], F32)
            for c in range(C3):
                scal = ext[:, c, p:p + 1]
                if BALANCE[p * C3 + c] == "v":
                    nc.vector.tensor_scalar_mul(out=st[:, c, :], in0=ps,
                                                scalar1=scal)
                else:
                    nc.scalar.mul(out=st[:, c, :], in_=ps, mul=scal)
            # DRAM row (within pair p) = 3*par + c: one contiguous 4608B
            # run per partition.
            nc.sync.dma_start(
                out=out[p * H:(p + 1) * H, :].rearrange(
                    "(par c) w -> par c w", c=C3),
                in_=st,
            )


LAST_RESULTS = None  # BassKernelResults of the most recent kernel() call


def _pack_inputs(labels: np.ndarray, log_weight: np.ndarray) -> np.ndarray:
    """[12, 399] per-core f32 pack: labels | logw | grid | identity."""
    pk = np.empty((PAIRS, PK_N), dtype=np.float32)
    pk[:, PK_LAB:PK_LAB + 2] = labels
    pk[:, PK_LW] = np.float32(log_weight).reshape(())
    pk[:, PK_GRID:PK_GRID + W] = (np.arange(W, dtype=np.float32)
                                  / np.float32(W))[None, :]
    pk[:, PK_ID:PK_ID + PAIRS] = np.eye(PAIRS, dtype=np.float32)
    return pk


def kernel(x: np.ndarray, labels: np.ndarray,
           log_weight: np.ndarray, **run_kwargs) -> np.ndarray:
    global LAST_RESULTS
    del x  # only its (hardcoded) shape matters
    import ml_dtypes
    nc = build_bass()
    labels = np.ascontiguousarray(labels, dtype=np.float32)
    bigsel = np.kron(np.eye(PAIRS, dtype=np.float32),
                     np.ones((1, P), dtype=np.float32)
                     ).astype(ml_dtypes.bfloat16)
    in_maps = [
        {
            "pack1": _pack_inputs(
                labels[i * BPC:(i + 1) * BPC].reshape(PAIRS, 2), log_weight),
            "bigsld": bigsel,
        }
        for i in range(N_CORES)
    ]
    res = run_bass_kernel_spmd(nc, in_maps, core_ids=list(range(N_CORES)),
                               **run_kwargs)
    LAST_RESULTS = res
    outs = [r["out"].reshape(BPC, NCLS, H, W) for r in res.results]
    return np.concatenate(outs, axis=0)


if __name__ == "__main__":
    rng = np.random.default_rng(0)
    x = rng.standard_normal((B, CH, H, W), dtype=np.float32)
    labels = rng.random((B, 2 * NCLS), dtype=np.float32)
    lw = rng.random((1, 1, 1, 1), dtype=np.float32)
    y = kernel(x=x, labels=labels, log_weight=lw)
    print(y.shape, y.dtype, y.min(), y.max())


# revision 11
# speedup vs baseline: 1.3007x; 1.0323x over previous
"""Trainium2 Bass kernel for nn_HeatmapLayer: separable Gaussian heatmaps.

Reference math (per batch b, class c):
    mx = labels[b, 2c] * H ; my = labels[b, 2c+1] * W          (H = W = 384)
    sigma = H * exp(log_weight)
    dx2[h] = (h - mx)^2 / sigma        ; normalized by its min over h
    dy2[w] = (w - my)^2 / (20 * sigma) ; normalized by its min over w
    out[b,c,h,w] = exp(-0.5*(dx2[h] + dy2[w])) = ex[h] * ey[w]

Each (b,c) heatmap is a rank-1 outer product of two 384-length
profiles; 2 batches x 6 classes = 12 pairs per core (batch-parallel
over 8 cores).  The kernel is output-DMA-bound: 7.08MB/core over 16
DMA engines at ~360GB/s aggregate = ~19.7us, so everything else is
organized to start that drain as early as possible and keep it
gap-free.

Key structure:
  * PE outer-product: ex/ey profiles are computed once on 12
    partitions; the idle PE engine broadcasts ey (bf16) to all 128
    partitions (one K=12 matmul per pair against a 0/1 block-selector)
    and each output chunk is produced by exactly ONE DVE/ACT op
    reading PSUM and scaling by the transposed x-profile EXT
    (per-partition scalar) -- the engine minimum per output element.
  * The start-of-kernel all-engine barrier is rebuilt WITHOUT the
    GpSimd/Pool engine: its Q7 cores take ~6us to boot and nothing in
    this kernel uses them (constants arrive as host-provided DMA
    inputs; no float activation biases -> no const-AP reads, which are
    the only thing the stock barrier protects here).
  * ACT's Exp table (~1.3us load) is warmed by a dummy Exp on a
    memset tile before the inputs even arrive.
  * All f32 inputs ride in ONE packed [12, 399] DMA (labels, logw,
    grid, identity); the bf16 block-selector is a second DMA on the
    ACT HWDGE queue.
  * The per-axis min of (h-m)^2 is computed EXACTLY from the labels
    alone with the +-2^23 round-to-integer trick + clamp (tiny DVE
    ops hidden under the ACT Squares) instead of a 384-wide reduce.
  * Output rows are staged as h = 3*par + c so each SBUF partition is
    one contiguous 4608B DRAM run.  EXT[par,c,p] = ex_p[3*par+c] comes
    from 3 stride-3 PE transposes.
"""

import numpy as np
from contextlib import ExitStack

import concourse.bacc as bacc
import concourse.bass as bass
import concourse.tile as tile
from concourse import mybir
from concourse.bass_utils import run_bass_kernel_spmd

B, CH, H, W = 16, 3, 384, 384
NCLS = 6
N_CORES = 8
BPC = B // N_CORES            # batches per core = 2
PAIRS = BPC * NCLS            # (b,c) pairs per core = 12
P = 128
C3 = H // P                   # 3 chunks of 128 rows
LN_H = float(np.log(H))
RND = 12582912.0              # 1.5 * 2^23: add+subtract rounds to integer
F32 = mybir.dt.float32
BF16 = mybir.dt.bfloat16
AF = mybir.ActivationFunctionType
ALU = mybir.AluOpType

# packed f32 input layout: [labels(2) | logw(1) | grid(384) | ident(12)]
PK_LAB, PK_LW, PK_GRID, PK_ID = 0, 2, 3, 387
PK_N = PK_ID + PAIRS

# engine for the 36 finals, by flat index (p*3+c): DVE 18 / ACT 18.
BALANCE = ("vav" + "ava") * 6


def _barrier_without_pool(self, *, sem_only: bool = False):
    engines = [e for e in self.engines if e != mybir.EngineType.Pool]
    if sem_only:
        for inst in self._sem_only_all_engine_barrier_insts("aeb"):
            self.engines[inst.engine].add_instruction(inst)
    else:
        self.multi_engine_barrier(engines)


def build_bass() -> bass.Bass:
    orig_barrier = bass.Bass.all_engine_barrier
    bass.Bass.all_engine_barrier = _barrier_without_pool
    try:
        nc = bacc.Bacc("TRN2", target_bir_lowering=False, debug=False,
                       num_devices=N_CORES)
        _build_body(nc)
        nc.finalize()
    finally:
        bass.Bass.all_engine_barrier = orig_barrier
    return nc


def _build_body(nc) -> None:
    pack1 = nc.dram_tensor("pack1", [PAIRS, PK_N], F32, kind="ExternalInput")
    bigsld = nc.dram_tensor("bigsld", [PAIRS, PAIRS * P], BF16,
                            kind="ExternalInput")
    out = nc.dram_tensor("out", [PAIRS * H, W], F32, kind="ExternalOutput")

    with ExitStack() as ctx:
        tc = ctx.enter_context(tile.TileContext(nc))
        singles = ctx.enter_context(tc.tile_pool(name="singles", bufs=1))
        psT = ctx.enter_context(tc.tile_pool(name="psT", bufs=2,
                                             space="PSUM"))
        psB = ctx.enter_context(tc.tile_pool(name="psB", bufs=6,
                                             space="PSUM"))
        stage = ctx.enter_context(tc.tile_pool(name="stage", bufs=12))

        # ---- input DMAs, one per HWDGE queue ------------------------------
        pk = singles.tile([PAIRS, PK_N], F32)
        nc.sync.dma_start(out=pk, in_=pack1[:, :])
        bigsel = singles.tile([PAIRS, PAIRS * P], BF16)
        nc.scalar.dma_start(out=bigsel, in_=bigsld[:, :])

        lab = pk[:, PK_LAB:PK_LAB + 2]          # (mx, my)/H in [0,1)
        lwb = pk[:, PK_LW:PK_LW + 1]
        iog = pk[:, PK_GRID:PK_GRID + W]        # j/384 grid
        ident = pk[:, PK_ID:PK_ID + PAIRS]

        # ---- warm the ACT Exp table before inputs arrive ------------------
        zz = singles.tile([PAIRS, 2], F32)
        nc.vector.memset(zz, 0.0)
        warm = singles.tile([PAIRS, 1], F32)
        nc.scalar.activation(out=warm, in_=zz[:, 0:1], func=AF.Exp,
                             bias=zz[:, 1:2], scale=1.0)
        zcol = zz[:, 1:2]                        # zeros bias AP

        # ---- per-pair scalars (DVE, tiny; hidden under ACT Squares) -------
        # inv_s = exp(-lw - lnH); grid is j/384 so sq=(lab-j/384)^2 and the
        # 384^2 folds into the exp scales.
        nlw = singles.tile([PAIRS, 1], F32)
        nc.vector.tensor_scalar(out=nlw, in0=lwb, scalar1=-1.0,
                                scalar2=-LN_H, op0=ALU.mult, op1=ALU.add)
        inv_s = singles.tile([PAIRS, 1], F32)
        nc.scalar.activation(out=inv_s, in_=nlw, func=AF.Exp,
                             bias=zcol, scale=1.0)
        sc2 = singles.tile([PAIRS, 2], F32)     # exp scale (negative)
        nc.vector.tensor_scalar_mul(out=sc2[:, 0:1], in0=inv_s,
                                    scalar1=-0.5 * float(H) * float(H))
        nc.vector.tensor_scalar_mul(out=sc2[:, 1:2], in0=inv_s,
                                    scalar1=-0.025 * float(H) * float(H))
        posc2 = singles.tile([PAIRS, 2], F32)   # |sc2| / 384^2
        nc.vector.tensor_scalar_mul(out=posc2[:, 0:1], in0=inv_s,
                                    scalar1=0.5)
        nc.vector.tensor_scalar_mul(out=posc2[:, 1:2], in0=inv_s,
                                    scalar1=0.025)

        # exact min of (h-m)^2 over integer h in [0,383], from labels only:
        # h* = clamp(round(m), max 383), min = (m-h*)^2  (in grid^2 units)
        m2c = singles.tile([PAIRS, 2], F32)     # m = 384*lab
        nc.vector.tensor_scalar_mul(out=m2c, in0=lab, scalar1=float(H))
        t1 = singles.tile([PAIRS, 2], F32)
        nc.vector.tensor_scalar_add(out=t1, in0=m2c, scalar1=RND)
        rr = singles.tile([PAIRS, 2], F32)      # round(m) (half-to-even)
        nc.vector.tensor_scalar_add(out=rr, in0=t1, scalar1=-RND)
        rc = singles.tile([PAIRS, 2], F32)      # clamp to grid max
        nc.vector.tensor_scalar_min(out=rc, in0=rr, scalar1=float(H - 1))
        dd = singles.tile([PAIRS, 2], F32)
        nc.vector.tensor_sub(out=dd, in0=m2c, in1=rc)
        mn = singles.tile([PAIRS, 2], F32)
        nc.vector.tensor_mul(out=mn, in0=dd, in1=dd)
        nb = singles.tile([PAIRS, 2], F32)      # exp bias: -sc2*min >= 0
        nc.vector.tensor_mul(out=nb, in0=mn, in1=posc2)

        # ---- profiles: y first (feeds the matmuls) ------------------------
        sqxy = singles.tile([PAIRS, 2, W], F32)
        nc.scalar.activation(out=sqxy[:, 1, :], in_=iog, func=AF.Square,
                             bias=lab[:, 1:2], scale=-1.0)
        ey = singles.tile([PAIRS, W], BF16)     # y profile (matmul rhs)
        nc.scalar.activation(out=ey, in_=sqxy[:, 1, :], func=AF.Exp,
                             bias=nb[:, 1:2], scale=sc2[:, 1:2])
        nc.scalar.activation(out=sqxy[:, 0, :], in_=iog, func=AF.Square,
                             bias=lab[:, 0:1], scale=-1.0)
        ex = singles.tile([PAIRS, W], F32)      # x profile (to transpose)
        nc.scalar.activation(out=ex, in_=sqxy[:, 0, :], func=AF.Exp,
                             bias=nb[:, 0:1], scale=sc2[:, 0:1])

        # ---- first matmul can go as soon as ey lands ----------------------
        def pair_matmul(p):
            ps = psB.tile([P, W], F32)
            nc.tensor.matmul(ps, bigsel[:, p * P:(p + 1) * P], ey,
                             start=True, stop=True)
            return ps

        ps0 = pair_matmul(0)

        # ---- EXT[par, c, p] = ex_p[3*par + c] via 3 strided PE transposes -
        exr = ex[:, :].rearrange("p (h c) -> p c h", c=C3)
        ext = singles.tile([P, C3, PAIRS], F32)
        for c in range(C3):
            pt = psT.tile([P, PAIRS], F32)
            nc.tensor.transpose(pt, exr[:, c, :], ident)
            nc.vector.tensor_copy(out=ext[:, c, :], in_=pt)

        # ---- main loop: one final op per output chunk, then one DMA -------
        # Pair 0 streams out as 3 per-chunk DMAs so the drain starts right
        # after the first final; later pairs go as whole 576KB DMAs.
        for p in range(PAIRS):
            ps = ps0 if p == 0 else pair_matmul(p)
            st = stage.tile([P, C3, W], F32)
            od = out[p * H:(p + 1) * H, :].rearrange(
                "(par c) w -> par c w", c=C3)
            for c in range(C3):
                scal = ext[:, c, p:p + 1]
                if BALANCE[p * C3 + c] == "v":
                    nc.vector.tensor_scalar_mul(out=st[:, c, :], in0=ps,
                                                scalar1=scal)
                else:
                    nc.scalar.mul(out=st[:, c, :], in_=ps, mul=scal)
                if p == 0:
                    nc.sync.dma_start(out=od[:, c, :], in_=st[:, c, :])
            # DRAM row (within pair p) = 3*par + c: one contiguous 4608B
            # run per partition.
            if p > 0:
                nc.sync.dma_start(out=od, in_=st)


LAST_RESULTS = None  # BassKernelResults of the most recent kernel() call


def _pack_inputs(labels: np.ndarray, log_weight: np.ndarray) -> np.ndarray:
    """[12, 399] per-core f32 pack: labels | logw | grid | identity."""
    pk = np.empty((PAIRS, PK_N), dtype=np.float32)
    pk[:, PK_LAB:PK_LAB + 2] = labels
    pk[:, PK_LW] = np.float32(log_weight).reshape(())
    pk[:, PK_GRID:PK_GRID + W] = (np.arange(W, dtype=np.float32)
                                  / np.float32(W))[None, :]
    pk[:, PK_ID:PK_ID + PAIRS] = np.eye(PAIRS, dtype=np.float32)
    return pk


def kernel(x: np.ndarray, labels: np.ndarray,
           log_weight: np.ndarray, **run_kwargs) -> np.ndarray:
    global LAST_RESULTS
    del x  # only its (hardcoded) shape matters
    import ml_dtypes
    nc = build_bass()
    labels = np.ascontiguousarray(labels, dtype=np.float32)
    bigsel = np.kron(np.eye(PAIRS, dtype=np.float32),
                     np.ones((1, P), dtype=np.float32)
                     ).astype(ml_dtypes.bfloat16)
    in_maps = [
        {
            "pack1": _pack_inputs(
                labels[i * BPC:(i + 1) * BPC].reshape(PAIRS, 2), log_weight),
            "bigsld": bigsel,
        }
        for i in range(N_CORES)
    ]
    res = run_bass_kernel_spmd(nc, in_maps, core_ids=list(range(N_CORES)),
                               **run_kwargs)
    LAST_RESULTS = res
    outs = [r["out"].reshape(BPC, NCLS, H, W) for r in res.results]
    return np.concatenate(outs, axis=0)


if __name__ == "__main__":
    rng = np.random.default_rng(0)
    x = rng.standard_normal((B, CH, H, W), dtype=np.float32)
    labels = rng.random((B, 2 * NCLS), dtype=np.float32)
    lw = rng.random((1, 1, 1, 1), dtype=np.float32)
    y = kernel(x=x, labels=labels, log_weight=lw)
    print(y.shape, y.dtype, y.min(), y.max())
